# revision 26
# baseline (speedup 1.0000x reference)
"""
MultiHeadAttention (B=4, S=2048, D=768, H=12, dk=64) on 8 TRN2 NeuronCores,
with host-side live-query compaction.

The reference mask kills whole QUERY rows (softmax of a constant row ->
uniform -> output computable on host as (mean_k V) @ Wo^T + bo). Keys are
never masked. So the device only processes the ~50% live query rows:
host compacts q[b] to the live rows (padded to SQ, a multiple of 128),
the device computes attention for SQ queries against all 2048 keys, and
the host scatters results back + fills dead rows with the closed form.

Sharding: core c -> (batch b = c//2, head-group g = c%2 of 6 heads).
Per core, for its (b, g):
    Q^T/K^T = Wx_g @ x[b]^T   (dout on partitions)
    Vaug    = [v[b] @ Wv_g^T | ones]  (65 cols/head; ones col gives the
              softmax denominator for free in the AV matmul)
    E^T     = exp(scores^T / 8)  (no max subtraction: |scores|/8 <= ~7)
    out^T_h = Vaug_h^T @ E^T_h  (rows 0..63 = unnorm out^T, row 64 = den)
    concat^T normalized via reciprocal_approx_fast + gpsimd broadcast
    partial_out = concat^T.T @ Wo_g^T + bo/2
Host sums the two head-group partials per batch and scatters to live rows.

dtypes: all matmuls bf16; f32 PSUM accumulation and normalization.
"""

import numpy as np
import ml_dtypes

import concourse.bass as bass
import concourse.tile as tile
from concourse import bacc, mybir
from concourse.bass_utils import run_bass_kernel_spmd

F32 = mybir.dt.float32
BF16 = mybir.dt.bfloat16
AF = mybir.ActivationFunctionType
OP = mybir.AluOpType

B, S, D, H, DK = 4, 2048, 768, 12, 64
NCORES = 8
HG = 6            # heads per core
DH = HG * DK      # 384 head dims per core
P = 128
DC = D // P       # 6 contraction chunks for the input projections
MC = DH // P      # 3 dout chunks for Q^T/K^T/concatT
SC = S // P       # 16 key chunks
KST = 512         # key-side tile (K/V projection granularity)
NKST = S // KST   # 4


def build_nc(sq=S):
    """SPMD single-core program for SQ (padded live) queries x S keys.
    sq must be a multiple of 128 and >= 512."""
    assert sq % 64 == 0 and 512 <= sq <= S
    # query tiles: full 512s plus one remainder (a multiple of 64)
    qtiles = []
    off = 0
    while off < sq:
        st = min(512, sq - off)
        qtiles.append((off, st))
        off += st
    NQT = len(qtiles)
    sq_out = ((sq + P - 1) // P) * P   # out rows padded to 128-chunks

    nc = bacc.Bacc("TRN2", target_bir_lowering=False, debug=False,
                   enable_asserts=True, num_devices=NCORES)

    qT = nc.dram_tensor("qT", [D, sq], BF16, kind="ExternalInput").ap()
    kT = nc.dram_tensor("kT", [D, S], BF16, kind="ExternalInput").ap()
    vT = nc.dram_tensor("vT", [D, S], BF16, kind="ExternalInput").ap()
    wqT = nc.dram_tensor("wqT", [D, DH], BF16, kind="ExternalInput").ap()
    wkT = nc.dram_tensor("wkT", [D, DH], BF16, kind="ExternalInput").ap()
    wvT = nc.dram_tensor("wvT", [D, DH], BF16, kind="ExternalInput").ap()
    woT = nc.dram_tensor("woT", [DH, D], BF16, kind="ExternalInput").ap()
    bqg = nc.dram_tensor("bqg", [P, MC], F32, kind="ExternalInput").ap()
    bkg = nc.dram_tensor("bkg", [P, MC], F32, kind="ExternalInput").ap()
    bvg = nc.dram_tensor("bvg", [P, DH], F32, kind="ExternalInput").ap()
    bog = nc.dram_tensor("bog", [P, D], F32, kind="ExternalInput").ap()
    out = nc.dram_tensor("out", [sq_out, D], F32, kind="ExternalOutput").ap()

    qT_r = qT.rearrange("(dc p) s -> p dc s", p=P)
    kT_r = kT.rearrange("(dc p) s -> p dc s", p=P)
    vT_r = vT.rearrange("(dc p) s -> p dc s", p=P)

    with tile.TileContext(nc) as tc:
        with (
            tc.tile_pool(name="consts", bufs=1) as consts,
            tc.tile_pool(name="persist", bufs=1) as persist,
            tc.tile_pool(name="staging", bufs=3) as staging,
            tc.tile_pool(name="et", bufs=6 if sq <= 1280 else 5) as etp,
            tc.tile_pool(name="bc", bufs=4) as bcp,
            tc.tile_pool(name="outp", bufs=4) as outp,
            tc.tile_pool(name="ps", bufs=4, space="PSUM") as psp,
            tc.tile_pool(name="ps_s", bufs=2, space="PSUM") as psps,
        ):
            # ---- constants ----
            wq_sb = consts.tile([P, DC, DH], BF16)
            wk_sb = consts.tile([P, DC, DH], BF16)
            wv_sb = consts.tile([P, DC, DH], BF16)
            wo_sb = consts.tile([P, MC, D], BF16)
            bq_sb = consts.tile([P, MC], F32)
            bk_sb = consts.tile([P, MC], F32)
            bv_sb = consts.tile([P, DH], F32)
            bo_sb = consts.tile([P, D], F32)
            nc.sync.dma_start(out=wk_sb, in_=wkT.rearrange("(c p) m -> p c m", p=P))
            nc.sync.dma_start(out=bk_sb, in_=bkg)

            def emit_q_consts():
                nc.sync.dma_start(
                    out=wq_sb, in_=wqT.rearrange("(c p) m -> p c m", p=P))
                nc.sync.dma_start(out=bq_sb, in_=bqg)

            def emit_late_consts():
                nc.sync.dma_start(
                    out=wv_sb, in_=wvT.rearrange("(c p) m -> p c m", p=P))
                nc.sync.dma_start(out=bv_sb, in_=bvg)
                nc.sync.dma_start(
                    out=wo_sb, in_=woT.rearrange("(c p) e -> p c e", p=P))
                nc.sync.dma_start(out=bo_sb, in_=bog)

            # ---- persistent intermediates ----
            QT = persist.tile([P, MC, sq], BF16)      # head h at [hp:hp+64, h//2]
            KT = persist.tile([P, MC, S], BF16)
            Vaug = persist.tile([P, SC, HG, 2 * DK], BF16)
            concatT = persist.tile([P, MC, sq_out], BF16)
            if sq_out > sq:
                nc.gpsimd.memset(concatT[:, :, sq:], 0.0)
            nc.gpsimd.memset(Vaug[:, :, :, DK + 1:], 0.0)
            nc.gpsimd.memset(Vaug[:, :, :, DK:DK + 1], 1.0)

            # ---- emit helpers ----
            def stage(src_r, lo, width, name):
                xt = staging.tile([P, DC, 512], BF16, tag="stage", name=name)
                nc.sync.dma_start(out=xt[:, :, :width], in_=src_r[:, :, lo:lo + width])
                return xt

            def emit_kproj(st, xt=None):
                ssl = slice(st * KST, (st + 1) * KST)
                if xt is None:
                    xt = stage(kT_r, st * KST, KST, "kt")
                for m in range(MC):
                    ps = psp.tile([P, 512], F32, tag="ps", name="ps_p")
                    for dc in range(DC):
                        nc.tensor.matmul(
                            ps[:, :KST],
                            lhsT=wk_sb[:, dc, m * P:(m + 1) * P],
                            rhs=xt[:, dc, :KST],
                            start=(dc == 0), stop=(dc == DC - 1),
                        )
                    nc.vector.tensor_scalar_add(
                        KT[:, m, ssl], ps[:, :KST], bk_sb[:, m:m + 1],
                    )

            def emit_qproj(qti, xt=None):
                qoff, qst = qtiles[qti]
                ssl = slice(qoff, qoff + qst)
                if xt is None:
                    xt = stage(qT_r, qoff, qst, "qt")
                for m in range(MC):
                    ps = psp.tile([P, 512], F32, tag="ps", name="ps_p")
                    for dc in range(DC):
                        nc.tensor.matmul(
                            ps[:, :qst],
                            lhsT=wq_sb[:, dc, m * P:(m + 1) * P],
                            rhs=xt[:, dc, :qst],
                            start=(dc == 0), stop=(dc == DC - 1),
                        )
                    nc.vector.tensor_scalar_add(
                        QT[:, m, ssl], ps[:, :qst], bq_sb[:, m:m + 1],
                    )

            def emit_vproj(st):
                ssl = slice(st * KST, (st + 1) * KST)
                vt = staging.tile([P, DC, 512], BF16, tag="stage", name="vt")
                nc.sync.dma_start(out=vt[:, :, :KST], in_=vT_r[:, :, ssl])
                for sc4 in range(KST // P):
                    kcg = st * (KST // P) + sc4
                    psv = psp.tile([P, 512], F32, tag="ps", name="ps_v")
                    for dc in range(DC):
                        nc.tensor.matmul(
                            psv[:, :DH],
                            lhsT=vt[:, dc, sc4 * P:(sc4 + 1) * P],
                            rhs=wv_sb[:, dc, :],
                            start=(dc == 0), stop=(dc == DC - 1),
                        )
                    nc.vector.tensor_tensor(
                        out=Vaug[:, kcg, :, 0:DK],
                        in0=psv[:, :DH].rearrange("p (h d) -> p h d", h=HG),
                        in1=bv_sb.rearrange("p (h d) -> p h d", h=HG),
                        op=OP.add,
                    )

            def alloc_et():
                return etp.tile([P, SC * 512], BF16, tag="et", name="et")

            def emit_scores_pair_part(h0, qti, ET0, ET1, kc_starts, gk):
                """Per group of gk key-chunks: interleave the two heads'
                score matmuls (different PE row groups -> LDW overlap),
                then one exp per head over the whole psum tile."""
                qoff, qst = qtiles[qti]
                qsl = slice(qoff, qoff + qst)
                hc = h0 // 2
                for kc0 in kc_starts:
                    ps0 = psps.tile([P, 1024], F32, tag="ps_s", name="ps_s0")
                    ps1 = psps.tile([P, 1024], F32, tag="ps_s", name="ps_s1")
                    for j in range(gk):
                        kc = kc0 + j
                        jsl = slice(j * qst, (j + 1) * qst)
                        nc.tensor.matmul(
                            ps0[:, jsl],
                            lhsT=KT[0:DK, hc, kc * P:(kc + 1) * P],
                            rhs=QT[0:DK, hc, qsl],
                            start=True, stop=True,
                            tile_position=(0, 0),
                        )
                        nc.tensor.matmul(
                            ps1[:, jsl],
                            lhsT=KT[DK:P, hc, kc * P:(kc + 1) * P],
                            rhs=QT[DK:P, hc, qsl],
                            start=True, stop=True,
                            tile_position=(DK, 0),
                        )
                    nc.scalar.activation(
                        out=ET0[:, kc0 * qst:(kc0 + gk) * qst],
                        in_=ps0[:, :gk * qst], func=AF.Exp, scale=0.125,
                    )
                    nc.scalar.activation(
                        out=ET1[:, kc0 * qst:(kc0 + gk) * qst],
                        in_=ps1[:, :gk * qst], func=AF.Exp, scale=0.125,
                    )

            def _gk(qst):
                g = 1
                while g * 2 <= min(16, 1024 // qst):
                    g *= 2
                return max(2, g) if qst <= 512 else 2

            def emit_scores_pair(h0, qti, mid_cb=None):
                """Emit the pair's score matmuls in two halves with an
                optional callback between them, so big AV blocks land at
                few fixed points inside the scores stream (fewer PE
                lhsT-stream switches => better LDWEIGHTS pull-ahead)."""
                ET0, ET1 = alloc_et(), alloc_et()
                qst = qtiles[qti][1]
                gk = _gk(qst)
                starts = list(range(0, SC, gk))
                half = len(starts) // 2
                emit_scores_pair_part(h0, qti, ET0, ET1, starts[:half], gk)
                if mid_cb is not None:
                    mid_cb()
                emit_scores_pair_part(h0, qti, ET0, ET1, starts[half:], gk)
                return ET0, ET1

            def emit_av(h, qti, ET):
                hp = (h % 2) * DK
                hc = (h // 2)
                qoff, qst = qtiles[qti]
                qsl = slice(qoff, qoff + qst)
                ps_o = psp.tile([P, 512], F32, tag="ps", name="ps_o")
                for kc in range(SC):
                    nc.tensor.matmul(
                        ps_o[0:DK + 1, :qst],
                        lhsT=Vaug[:, kc, h, 0:DK + 1],  # V | ones
                        rhs=ET[:, kc * qst:(kc + 1) * qst],
                        start=(kc == 0), stop=(kc == SC - 1),
                    )
                dn = bcp.tile([P, 512], F32, tag="bc", name="dn")
                bc = bcp.tile([P, 512], F32, tag="bc", name="bc")
                nc.vector.tensor_copy(out=dn[0:1, :qst], in_=ps_o[DK:DK + 1, :qst])
                nc.vector.reciprocal_approx_fast(out=bc[0:1, :qst], in_=dn[0:1, :qst])
                nc.gpsimd.partition_broadcast(bc[0:DK, :qst], bc[0:1, :qst])
                nc.vector.tensor_tensor(
                    out=concatT[hp:hp + DK, hc, qsl],
                    in0=ps_o[0:DK, :qst],
                    in1=bc[0:DK, :qst],
                    op=OP.mult,
                )

            def emit_outproj(sc):
                osb = outp.tile([P, D], F32, tag="o", name="osb")
                for n in range(D // DH):
                    nsl = slice(n * DH, (n + 1) * DH)
                    ps_f = psp.tile([P, 512], F32, tag="ps", name="ps_f")
                    for c in range(MC):
                        nc.tensor.matmul(
                            ps_f[:, :DH],
                            lhsT=concatT[:, c, sc * P:(sc + 1) * P],
                            rhs=wo_sb[:, c, nsl],
                            start=(c == 0), stop=(c == MC - 1),
                        )
                    nc.vector.tensor_tensor(
                        out=osb[:, nsl], in0=ps_f[:, :DH], in1=bo_sb[:, nsl],
                        op=OP.add,
                    )
                nc.sync.dma_start(out=out[sc * P:(sc + 1) * P, :], in_=osb)

            # ---- emission order: start ACT exp work early; K/V/Q
            # projections fill PE while ACT chews the first heads' exps.
            # kT slab + wk go down the DMA queue first so K-proj matmuls
            # start ASAP; all 6 heads' qt0 scores stream in the prologue so
            # ACT has deep exp backlog while the PE does K/V projections.
            npre = 6 if sq <= 1280 else 4
            kt0 = stage(kT_r, 0, KST, "kt")
            emit_q_consts()
            qt0 = stage(qT_r, 0, qtiles[0][1], "qt")
            emit_kproj(0, kt0)
            emit_qproj(0, qt0)
            ets0 = {h: alloc_et() for h in range(npre)}
            for st in range(1, NKST):
                for h0 in range(0, npre, 2):
                    emit_scores_pair_part(
                        h0, 0, ets0[h0], ets0[h0 + 1],
                        range((st - 1) * (KST // P), st * (KST // P), 2), 2)
                emit_kproj(st)
                if st == 1:
                    # V/O weights go down the DMA queue after the
                    # latency-critical kt1 slab; V-proj has slack
                    emit_late_consts()
                emit_vproj(st - 1)
            for h0 in range(0, npre, 2):
                emit_scores_pair_part(
                    h0, 0, ets0[h0], ets0[h0 + 1],
                    range((NKST - 1) * (KST // P), SC, 2), 2)
            emit_vproj(NKST - 1)
            # ---- software-pipelined main loop: after emitting scores for
            # pair p, emit the AVs of the OLDEST pending pair, so the PE has
            # dense AV work while ACT chews pair p's exps. Q-projections and
            # per-qtile outprojs slot in as additional PE filler.
            if NQT > 1:
                emit_qproj(1)
            pending = [(h, 0, ets0.pop(h)) for h in range(npre)]
            pair_seq = [(hh, qti) for qti in range(NQT) for hh in range(0, HG, 2)
                        ][npre // 2:]
            done_av = {qti: 0 for qti in range(NQT)}

            deferred = []

            def drain_avs(n):
                for _ in range(n):
                    if not pending:
                        return
                    h, qti, et = pending.pop(0)
                    emit_av(h, qti, et)
                    done_av[qti] += 1
                    if done_av[qti] == HG:
                        if qti + 2 < NQT:
                            emit_qproj(qti + 2)
                        deferred.append(qti)

            def flush_outproj():
                while deferred:
                    qti = deferred.pop(0)
                    qoff, qst = qtiles[qti]
                    hi = min((qoff + qst + P - 1) // P, sq_out // P)
                    for sc in range(qoff // P, hi):
                        emit_outproj(sc)

            # outprojs are emitted AFTER the next pair's scores so the
            # ACT-feeding score matmuls never queue behind a dependency-free
            # out-projection block
            for hh, qti in pair_seq:
                drain_avs(2)
                e0, e1 = emit_scores_pair(hh, qti)
                flush_outproj()
                pending.append((hh, qti, e0))
                pending.append((hh + 1, qti, e1))
            drain_avs(len(pending))
            flush_outproj()

    nc.compile()
    return nc


def _compact(mask):
    """Per-batch live-row indices and the common padded length SQ."""
    live = [np.nonzero(np.asarray(mask[b]) != 0)[0] for b in range(B)]
    nmax = max((len(ix) for ix in live), default=0)
    sq = min(S, max(512, ((max(nmax, 1) + 63) // 64) * 64))
    return live, sq


def make_in_maps(q, k, v, Wq, bq, Wk, bk, Wv, bv, Wo, bo, live, sq):
    """Per-core input shards. Core c -> batch c//2, head-group c%2."""
    f32 = np.float32
    q, k, v = (np.asarray(x, f32) for x in (q, k, v))
    Wq, Wk, Wv, Wo = (np.asarray(x, f32) for x in (Wq, Wk, Wv, Wo))
    bq, bk, bv, bo = (np.asarray(x, f32) for x in (bq, bk, bv, bo))
    bf16 = ml_dtypes.bfloat16
    qTs, kTs, vTs = [], [], []
    for b in range(B):
        qc = np.zeros((sq, D), f32)
        n = len(live[b])
        qc[:n] = q[b][live[b]]
        qTs.append(np.ascontiguousarray(qc.T).astype(bf16))
        kTs.append(np.ascontiguousarray(k[b].T).astype(bf16))
        vTs.append(np.ascontiguousarray(v[b].T).astype(bf16))
    in_maps = []
    for c in range(NCORES):
        b, g = c // 2, c % 2
        sl = slice(g * DH, (g + 1) * DH)
        in_maps.append({
            "qT": qTs[b],
            "kT": kTs[b],
            "vT": vTs[b],
            "wqT": np.ascontiguousarray(Wq[sl, :].T).astype(bf16),
            "wkT": np.ascontiguousarray(Wk[sl, :].T).astype(bf16),
            "wvT": np.ascontiguousarray(Wv[sl, :].T).astype(bf16),
            "woT": np.ascontiguousarray(Wo[:, sl].T).astype(bf16),
            "bqg": np.ascontiguousarray(bq[sl].reshape(MC, P).T),
            "bkg": np.ascontiguousarray(bk[sl].reshape(MC, P).T),
            "bvg": np.broadcast_to(bv[sl], (P, DH)).copy(),
            "bog": np.broadcast_to(bo * 0.5, (P, D)).copy(),
        })
    return in_maps


def combine_outputs(core_outs, live, v, mask, Wv, bv, Wo, bo):
    """Sum head-group partials, scatter to live rows; dead rows exact."""
    f32 = np.float32
    v = np.asarray(v, f32)
    mask = np.asarray(mask)
    Wv, Wo = np.asarray(Wv, f32), np.asarray(Wo, f32)
    bv, bo = np.asarray(bv, f32), np.asarray(bo, f32)
    out = np.empty((B, S, D), f32)
    for b in range(B):
        n = len(live[b])
        if n:
            s = core_outs[2 * b][:n] + core_outs[2 * b + 1][:n]
            out[b][live[b]] = s
        dead = mask[b] == 0
        if dead.any():
            vmean = v[b].mean(axis=0, dtype=np.float64).astype(f32)
            row = (vmean @ Wv.T + bv) @ Wo.T + bo
            out[b][dead] = row
    return out


_NC_CACHE = {}


def _get_nc(sq):
    if sq not in _NC_CACHE:
        _NC_CACHE[sq] = build_nc(sq)
    return _NC_CACHE[sq]


def run_on_hw(inputs, trace=False):
    live, sq = _compact(inputs["mask"])
    nc = _get_nc(sq)
    in_maps = make_in_maps(
        inputs["q"], inputs["k"], inputs["v"],
        inputs["Wq"], inputs["bq"], inputs["Wk"], inputs["bk"],
        inputs["Wv"], inputs["bv"], inputs["Wo"], inputs["bo"],
        live, sq,
    )
    res = run_bass_kernel_spmd(nc, in_maps, list(range(NCORES)), trace=trace)
    core_outs = [np.asarray(res.results[c]["out"]) for c in range(NCORES)]
    out = combine_outputs(core_outs, live, inputs["v"], inputs["mask"],
                          inputs["Wv"], inputs["bv"], inputs["Wo"], inputs["bo"])
    return out, res


def kernel(**inputs):
    out, _ = run_on_hw(inputs, trace=False)
    return out
